# revision 2
# baseline (speedup 1.0000x reference)
"""Trainium2 Bass kernel for Cross-MultiAttention.

Problem (hardcoded shapes):
  B=4, T=2048, S=2048, C=256, E=512, H=8 heads, D=64, SCALE=E**-0.5
  xe  = x @ w_in.T + b_in                  [B,T,C] -> [B,T,E]
  Q   = xe @ wq.T + bq;  K/V from context  [B,S,E]
  att = softmax(mask(QK^T * SCALE))        [B,H,T,S]
  out = (att @ V) @ w_out.T + b_out        -> [B,T,C]

Sharding: 8 cores, each handles 1024 query tokens (core c -> batch c//2,
token half c%2). K/V/context work is duplicated between the two cores of a
batch. All feature-major ("transposed") layouts on chip; host pre-packs
transposed bf16 operands.

Device algorithm per core (all matmuls bf16 in / fp32 PSUM accumulate):
  xeT  [E,Tc]  = w_inT.T @ xT   (+b_in via ACT bias)
  QT   [E,Tc]  = wqT.T @ xeT    (+bq)
  KT   [E,S]   = wkT.T @ ctxT   (+bk)
  Vaug [S,520] = ctxT.T @ wvT_aug (+bias row via K=1 ones matmul; col h*65+64
                 is the all-ones denominator column)
  per (head h, T-chunk t, key-tile j):
     logitsT[j*128:(j+1)*128, t*512:(t+1)*512] = KT_h_j.T @ QT_h_t   (PE)
     P = exp(SCALE * logitsT)                  (ACT, no max needed: |logits|<~1,
                                                masked handled multiplicatively)
     Pm = P * zT[j]                            (DVE; z = 1-mask in bf16)
     oav[0:65] += Vaug_j_h.T @ Pm              (PE; row 64 = softmax denominator)
  normalize: rec = 1/oav[64]; rb = broadcast(rec); ocat_h = oav[0:64]*rb
  finT [C,Tc] = w_outT.T @ ocat (+b_out via K=1 ones matmul)
"""

import numpy as np
import ml_dtypes

import concourse.bass as bass
import concourse.tile as tile
import concourse.mybir as mybir
from concourse.bacc import Bacc
from concourse.bass_utils import run_bass_kernel_spmd

BF16 = mybir.dt.bfloat16
F32 = mybir.dt.float32
AF = mybir.ActivationFunctionType

B, T, S, C, E, H = 4, 2048, 2048, 256, 512, 8
D = E // H
SCALE = float(E) ** -0.5
NCORES = 8
TPC = B * T // NCORES          # 1024 query tokens per core
NT = TPC // 512                # 2 T-chunks of 512
NJ = S // 128                  # 16 key tiles
KE = E // 128                  # 4
KC = C // 128                  # 2
MC = C // 128                  # 2 output c-tiles
HW = H * 65                    # 520 = V columns incl. denominator cols

_NC_CACHE = None


def _build_nc():
    nc = Bacc("TRN2", target_bir_lowering=False, debug=False)

    xT = nc.dram_tensor("xT", [128, KC, TPC], BF16, kind="ExternalInput")
    ctxT = nc.dram_tensor("ctxT", [128, KE, S], BF16, kind="ExternalInput")
    zT = nc.dram_tensor("zT", [128, NJ, TPC], BF16, kind="ExternalInput")
    w_inT = nc.dram_tensor("w_inT", [128, KC, E], BF16, kind="ExternalInput")
    wqT = nc.dram_tensor("wqT", [128, KE, E], BF16, kind="ExternalInput")
    wkT = nc.dram_tensor("wkT", [128, KE, E], BF16, kind="ExternalInput")
    wvT = nc.dram_tensor("wvT", [128, KE, HW], BF16, kind="ExternalInput")
    wvb = nc.dram_tensor("wvb", [1, HW], BF16, kind="ExternalInput")
    w_outT = nc.dram_tensor("w_outT", [64, H, C], BF16, kind="ExternalInput")
    wob = nc.dram_tensor("wob", [1, C], BF16, kind="ExternalInput")
    b_in = nc.dram_tensor("b_in", [128, KC * 2], F32, kind="ExternalInput")
    bq = nc.dram_tensor("bq", [128, KE], F32, kind="ExternalInput")
    bk = nc.dram_tensor("bk", [128, KE], F32, kind="ExternalInput")
    outT = nc.dram_tensor("outT", [128, MC, TPC], F32, kind="ExternalOutput")

    with tile.TileContext(nc) as tc:
        with tc.tile_pool(name="const", bufs=1) as cp, \
             tc.tile_pool(name="acts", bufs=1) as ap, \
             tc.tile_pool(name="pp", bufs=3) as pp, \
             tc.tile_pool(name="nrm", bufs=2) as nrm, \
             tc.tile_pool(name="mm", bufs=3, space="PSUM") as ps_mm, \
             tc.tile_pool(name="av", bufs=2, space="PSUM") as ps_av:

            # ---- persistent loads ----
            xT_sb = cp.tile([128, KC, TPC], BF16, tag="xT")
            nc.sync.dma_start(out=xT_sb, in_=xT[:, :, :])
            ctxT_sb = cp.tile([128, KE, S], BF16, tag="ctxT")
            nc.sync.dma_start(out=ctxT_sb, in_=ctxT[:, :, :])
            zT_sb = cp.tile([128, NJ, TPC], BF16, tag="zT")
            nc.sync.dma_start(out=zT_sb, in_=zT[:, :, :])
            w_inT_sb = cp.tile([128, KC, E], BF16, tag="w_inT")
            nc.sync.dma_start(out=w_inT_sb, in_=w_inT[:, :, :])
            wqT_sb = cp.tile([128, KE, E], BF16, tag="wqT")
            nc.sync.dma_start(out=wqT_sb, in_=wqT[:, :, :])
            wkT_sb = cp.tile([128, KE, E], BF16, tag="wkT")
            nc.sync.dma_start(out=wkT_sb, in_=wkT[:, :, :])
            wvT_sb = cp.tile([128, KE, HW], BF16, tag="wvT")
            nc.sync.dma_start(out=wvT_sb, in_=wvT[:, :, :])
            wvb_sb = cp.tile([1, HW], BF16, tag="wvb")
            nc.sync.dma_start(out=wvb_sb, in_=wvb[:, :])
            w_outT_sb = cp.tile([64, H, C], BF16, tag="w_outT")
            nc.sync.dma_start(out=w_outT_sb, in_=w_outT[:, :, :])
            wob_sb = cp.tile([1, C], BF16, tag="wob")
            nc.sync.dma_start(out=wob_sb, in_=wob[:, :])
            b_in_sb = cp.tile([128, KC * 2], F32, tag="b_in")
            nc.sync.dma_start(out=b_in_sb, in_=b_in[:, :])
            bq_sb = cp.tile([128, KE], F32, tag="bq")
            nc.sync.dma_start(out=bq_sb, in_=bq[:, :])
            bk_sb = cp.tile([128, KE], F32, tag="bk")
            nc.sync.dma_start(out=bk_sb, in_=bk[:, :])

            ones128 = cp.tile([1, 128], BF16, tag="ones128")
            nc.vector.memset(ones128, 1.0)
            onest = cp.tile([1, 512], BF16, tag="onest")
            nc.vector.memset(onest, 1.0)

            # ---- persistent activations ----
            xeT_sb = ap.tile([128, KE, TPC], BF16, tag="xeT")
            QT_sb = ap.tile([128, KE, TPC], BF16, tag="QT")
            KT_sb = ap.tile([128, KE, S], BF16, tag="KT")
            V_sb = ap.tile([128, NJ, HW], BF16, tag="V")
            ocat_sb = ap.tile([64, H, TPC], BF16, tag="ocat")
            fin_sb = ap.tile([128, MC, TPC], F32, tag="fin")

            # ---- proj_in: xeT[e, t] ----
            for m in range(KE):
                for t in range(NT):
                    p = ps_mm.tile([128, 512], F32, tag="mm")
                    for k in range(KC):
                        nc.tensor.matmul(
                            p[:, :],
                            w_inT_sb[:, k, m * 128:(m + 1) * 128],
                            xT_sb[:, k, t * 512:(t + 1) * 512],
                            start=(k == 0), stop=(k == KC - 1))
                    nc.scalar.activation(
                        out=xeT_sb[:, m, t * 512:(t + 1) * 512], in_=p[:, :],
                        func=AF.Identity, bias=b_in_sb[:, m:m + 1], scale=1.0)

            # ---- Q: QT[e, t] ----
            for m in range(KE):
                for t in range(NT):
                    p = ps_mm.tile([128, 512], F32, tag="mm")
                    for k in range(KE):
                        nc.tensor.matmul(
                            p[:, :],
                            wqT_sb[:, k, m * 128:(m + 1) * 128],
                            xeT_sb[:, k, t * 512:(t + 1) * 512],
                            start=(k == 0), stop=(k == KE - 1))
                    nc.scalar.activation(
                        out=QT_sb[:, m, t * 512:(t + 1) * 512], in_=p[:, :],
                        func=AF.Identity, bias=bq_sb[:, m:m + 1], scale=1.0)

            # ---- K: KT[e, s] ----
            for m in range(KE):
                for sc in range(S // 512):
                    p = ps_mm.tile([128, 512], F32, tag="mm")
                    for k in range(KE):
                        nc.tensor.matmul(
                            p[:, :],
                            wkT_sb[:, k, m * 128:(m + 1) * 128],
                            ctxT_sb[:, k, sc * 512:(sc + 1) * 512],
                            start=(k == 0), stop=(k == KE - 1))
                    nc.scalar.activation(
                        out=KT_sb[:, m, sc * 512:(sc + 1) * 512], in_=p[:, :],
                        func=AF.Identity, bias=bk_sb[:, m:m + 1], scale=1.0)

            # ---- V (token-major, head-interleaved + denominator col) ----
            for st in range(NJ):
                pv = ps_av.tile([128, HW], F32, tag="av")
                for k in range(KE):
                    nc.tensor.matmul(
                        pv[:, 0:512],
                        ctxT_sb[:, k, st * 128:(st + 1) * 128],
                        wvT_sb[:, k, 0:512],
                        start=(k == 0), stop=False)
                    nc.tensor.matmul(
                        pv[:, 512:HW],
                        ctxT_sb[:, k, st * 128:(st + 1) * 128],
                        wvT_sb[:, k, 512:HW],
                        start=(k == 0), stop=False)
                # bias row: adds bv to value cols and 1.0 to denominator cols
                nc.tensor.matmul(pv[:, 0:512], ones128[0:1, :], wvb_sb[0:1, 0:512],
                                 start=False, stop=True)
                nc.tensor.matmul(pv[:, 512:HW], ones128[0:1, :], wvb_sb[0:1, 512:HW],
                                 start=False, stop=True)
                nc.vector.tensor_copy(V_sb[:, st, :], pv[:, :])

            # ---- attention ----
            for h in range(H):
                et, bp = h // 2, 64 * (h % 2)
                for t in range(NT):
                    oav = ps_av.tile([65, 512], F32, tag="av")
                    for j in range(NJ):
                        pqk = ps_mm.tile([128, 512], F32, tag="mm")
                        nc.tensor.matmul(
                            pqk[:, :],
                            KT_sb[bp:bp + 64, et, j * 128:(j + 1) * 128],
                            QT_sb[bp:bp + 64, et, t * 512:(t + 1) * 512],
                            start=True, stop=True)
                        pe_t = pp.tile([128, 512], BF16, tag="pexp")
                        nc.scalar.activation(out=pe_t[:, :], in_=pqk[:, :],
                                             func=AF.Exp, scale=SCALE)
                        pm_t = pp.tile([128, 512], BF16, tag="pmask")
                        nc.vector.tensor_mul(
                            pm_t[:, :], pe_t[:, :],
                            zT_sb[:, j, t * 512:(t + 1) * 512])
                        nc.tensor.matmul(
                            oav[:, :],
                            V_sb[:, j, h * 65:(h + 1) * 65],
                            pm_t[:, :],
                            start=(j == 0), stop=(j == NJ - 1))
                    rec = nrm.tile([1, 512], F32, tag="rec")
                    nc.vector.reciprocal(rec[0:1, :], oav[64:65, :])
                    rb = nrm.tile([64, 512], F32, tag="rb")
                    nc.gpsimd.partition_broadcast(rb[:, :], rec[0:1, :])
                    nc.vector.tensor_mul(
                        ocat_sb[0:64, h, t * 512:(t + 1) * 512],
                        oav[0:64, :], rb[:, :])

            # ---- proj_out: finT[c, t] ----
            for m in range(MC):
                for t in range(NT):
                    pf = ps_mm.tile([128, 512], F32, tag="mm")
                    for h in range(H):
                        nc.tensor.matmul(
                            pf[:, :],
                            w_outT_sb[0:64, h, m * 128:(m + 1) * 128],
                            ocat_sb[0:64, h, t * 512:(t + 1) * 512],
                            start=(h == 0), stop=False)
                    nc.tensor.matmul(pf[:, :], wob_sb[0:1, m * 128:(m + 1) * 128],
                                     onest[0:1, :], start=False, stop=True)
                    nc.vector.tensor_copy(
                        fin_sb[:, m, t * 512:(t + 1) * 512], pf[:, :])

            nc.sync.dma_start(out=outT[:, :, :], in_=fin_sb)

    nc.finalize()
    return nc


def get_nc():
    global _NC_CACHE
    if _NC_CACHE is None:
        _NC_CACHE = _build_nc()
    return _NC_CACHE


def _pack(a, p=128):
    """[k*p, f...] -> [p, k, f...] C-contiguous."""
    k = a.shape[0] // p
    return np.ascontiguousarray(
        a.reshape(k, p, *a.shape[1:]).transpose(1, 0, *range(2, a.ndim + 1)))


def build_in_maps(x, context, pad_mask, w_in, b_in, wq, bq, wk, bk, wv, bv,
                  w_out, b_out):
    bf = ml_dtypes.bfloat16
    f32 = np.float32

    w_inT_p = _pack(w_in.T.astype(bf))                    # [128, 2, 512]
    wqT_p = _pack(wq.T.astype(bf))                        # [128, 4, 512]
    wkT_p = _pack(wk.T.astype(bf))
    wvT_aug = np.zeros((E, HW), dtype=bf)
    wvb_row = np.zeros((1, HW), dtype=bf)
    for h in range(H):
        wvT_aug[:, h * 65:h * 65 + 64] = wv.T[:, h * 64:(h + 1) * 64].astype(bf)
        wvb_row[0, h * 65:h * 65 + 64] = bv[h * 64:(h + 1) * 64].astype(bf)
        wvb_row[0, h * 65 + 64] = 1.0
    wvT_p = _pack(wvT_aug)                                # [128, 4, 520]
    w_outT_hm = np.ascontiguousarray(
        w_out.T.reshape(H, 64, C).transpose(1, 0, 2)).astype(bf)  # [64, 8, 256]
    wob_row = b_out[None, :].astype(bf)
    b_in_p = np.ascontiguousarray(b_in.reshape(KE, 128).T).astype(f32)
    bq_p = np.ascontiguousarray(bq.reshape(KE, 128).T).astype(f32)
    bk_p = np.ascontiguousarray(bk.reshape(KE, 128).T).astype(f32)

    z = (~pad_mask).astype(bf)                            # [B, T, S]

    in_maps = []
    for c in range(NCORES):
        b, th = c // 2, c % 2
        t0 = th * TPC
        xT_p = _pack(np.ascontiguousarray(x[b, t0:t0 + TPC, :].T).astype(bf))
        ctxT_p = _pack(np.ascontiguousarray(context[b].T).astype(bf))
        zT_p = _pack(np.ascontiguousarray(z[b, t0:t0 + TPC, :].T))
        in_maps.append({
            "xT": xT_p, "ctxT": ctxT_p, "zT": zT_p,
            "w_inT": w_inT_p, "wqT": wqT_p, "wkT": wkT_p, "wvT": wvT_p,
            "wvb": wvb_row, "w_outT": w_outT_hm, "wob": wob_row,
            "b_in": b_in_p, "bq": bq_p, "bk": bk_p,
        })
    return in_maps


def assemble_output(results):
    out = np.empty((B, T, C), dtype=np.float32)
    for c in range(NCORES):
        b, th = c // 2, c % 2
        t0 = th * TPC
        arr = results[c]["outT"]                          # [128, 2, 1024]
        ct = arr.transpose(1, 0, 2).reshape(C, TPC)       # [256, 1024]
        out[b, t0:t0 + TPC, :] = ct.T
    return out


def run(in_maps, **kw):
    return run_bass_kernel_spmd(get_nc(), in_maps, core_ids=list(range(NCORES)),
                                **kw)


def kernel(**inputs):
    in_maps = build_in_maps(
        np.asarray(inputs["x"]), np.asarray(inputs["context"]),
        np.asarray(inputs["pad_mask"]), np.asarray(inputs["w_in"]),
        np.asarray(inputs["b_in"]), np.asarray(inputs["wq"]),
        np.asarray(inputs["bq"]), np.asarray(inputs["wk"]),
        np.asarray(inputs["bk"]), np.asarray(inputs["wv"]),
        np.asarray(inputs["bv"]), np.asarray(inputs["w_out"]),
        np.asarray(inputs["b_out"]))
    res = run(in_maps)
    return assemble_output(res.results)


# revision 11
# speedup vs baseline: 172.4421x; 172.4421x over previous
"""Trainium2 Bass kernel for Cross-MultiAttention.

Problem (hardcoded shapes):
  B=4, T=2048, S=2048, C=256, E=512, H=8 heads, D=64, SCALE=E**-0.5
  xe  = x @ w_in.T + b_in                  [B,T,C] -> [B,T,E]
  Q   = xe @ wq.T + bq;  K/V from context  [B,S,E]
  att = softmax(mask(QK^T * SCALE))        [B,H,T,S]
  out = (att @ V) @ w_out.T + b_out        -> [B,T,C]

Sharding: 8 cores, each handles 1024 query tokens (core c -> batch c//2,
token half c%2). K/V/context work is duplicated between the two cores of a
batch. All feature-major ("transposed") layouts on chip; host pre-packs
transposed bf16 operands.

Device algorithm per core (all matmuls bf16 in / fp32 PSUM accumulate):
  xeT  [E,Tc]  = w_inT.T @ xT   (+b_in via ACT bias)
  QT   [E,Tc]  = wqT.T @ xeT    (+bq)
  KT   [E,S]   = wkT.T @ ctxT   (+bk)
  Vaug [S,520] = ctxT.T @ wvT_aug (+bias row via K=1 ones matmul; col h*65+64
                 is the all-ones denominator column)
  per (head h, T-chunk t, key-tile j):
     logitsT[j*128:(j+1)*128, t*512:(t+1)*512] = KT_h_j.T @ QT_h_t   (PE)
     P = exp(SCALE * logitsT)                  (ACT, no max needed: |logits|<~1,
                                                masked handled multiplicatively)
     Pm = P * zT[j]                            (DVE; z = 1-mask in bf16)
     oav[0:65] += Vaug_j_h.T @ Pm              (PE; row 64 = softmax denominator)
  normalize: rec = 1/oav[64]; rb = broadcast(rec); ocat_h = oav[0:64]*rb
  finT [C,Tc] = w_outT.T @ ocat (+b_out via K=1 ones matmul)
"""

import numpy as np
import ml_dtypes

import concourse.bass as bass
import concourse.tile as tile
import concourse.mybir as mybir
from concourse.bacc import Bacc
from concourse.bass_utils import run_bass_kernel_spmd

BF16 = mybir.dt.bfloat16
F32 = mybir.dt.float32
AF = mybir.ActivationFunctionType

B, T, S, C, E, H = 4, 2048, 2048, 256, 512, 8
D = E // H
SCALE = float(E) ** -0.5
NCORES = 8
TPC = B * T // NCORES          # 1024 query tokens per core
NT = TPC // 512                # 2 T-chunks of 512
NJ = S // 128                  # 16 key tiles
KE = E // 128                  # 4
KC = C // 128                  # 2
MC = C // 128                  # 2 output c-tiles
HW = H * 65                    # 520 = V columns incl. denominator cols

_NC_CACHE = None


def _build_nc():
    nc = Bacc("TRN2", target_bir_lowering=False, debug=False)

    xT = nc.dram_tensor("xT", [128, KC, TPC], BF16, kind="ExternalInput")
    ctxT = nc.dram_tensor("ctxT", [128, KE, S], BF16, kind="ExternalInput")
    zT = nc.dram_tensor("zT", [128, NJ, TPC], BF16, kind="ExternalInput")
    w_inT = nc.dram_tensor("w_inT", [128, KC, E], BF16, kind="ExternalInput")
    wqT = nc.dram_tensor("wqT", [128, KE, E], BF16, kind="ExternalInput")
    wkT = nc.dram_tensor("wkT", [128, KE, E], BF16, kind="ExternalInput")
    wvT = nc.dram_tensor("wvT", [128, KE, HW], BF16, kind="ExternalInput")
    wvb = nc.dram_tensor("wvb", [1, HW], BF16, kind="ExternalInput")
    w_outT = nc.dram_tensor("w_outT", [64, H, C], BF16, kind="ExternalInput")
    wob = nc.dram_tensor("wob", [1, C], BF16, kind="ExternalInput")
    b_in = nc.dram_tensor("b_in", [128, KC * 2], F32, kind="ExternalInput")
    bq = nc.dram_tensor("bq", [128, KE], F32, kind="ExternalInput")
    bk = nc.dram_tensor("bk", [128, KE], F32, kind="ExternalInput")
    outT = nc.dram_tensor("outT", [128, MC, TPC], F32, kind="ExternalOutput")

    with tile.TileContext(nc) as tc:
        with tc.tile_pool(name="const", bufs=1) as cp, \
             tc.tile_pool(name="acts", bufs=1) as ap, \
             tc.tile_pool(name="pp", bufs=6) as pp, \
             tc.tile_pool(name="nrm", bufs=2) as nrm, \
             tc.tile_pool(name="mm", bufs=3, space="PSUM") as ps_mm, \
             tc.tile_pool(name="av", bufs=2, space="PSUM") as ps_av:

            # ---- persistent loads (ordered so early-needed data lands first;
            # zT is big and only needed once attention starts) ----
            w_inT_sb = cp.tile([128, KC, E], BF16, tag="w_inT")
            nc.sync.dma_start(out=w_inT_sb, in_=w_inT[:, :, :])
            xT_sb = cp.tile([128, KC, TPC], BF16, tag="xT")
            nc.sync.dma_start(out=xT_sb, in_=xT[:, :, :])
            b_in_sb = cp.tile([128, KC * 2], F32, tag="b_in")
            nc.sync.dma_start(out=b_in_sb, in_=b_in[:, :])
            bq_sb = cp.tile([128, KE], F32, tag="bq")
            nc.sync.dma_start(out=bq_sb, in_=bq[:, :])
            bk_sb = cp.tile([128, KE], F32, tag="bk")
            nc.sync.dma_start(out=bk_sb, in_=bk[:, :])
            wqT_sb = cp.tile([128, KE, E], BF16, tag="wqT")
            nc.sync.dma_start(out=wqT_sb, in_=wqT[:, :, :])
            wkT_sb = cp.tile([128, KE, E], BF16, tag="wkT")
            nc.sync.dma_start(out=wkT_sb, in_=wkT[:, :, :])
            ctxT_sb = cp.tile([128, KE, S], BF16, tag="ctxT")
            nc.sync.dma_start(out=ctxT_sb, in_=ctxT[:, :, :])
            wvT_sb = cp.tile([128, KE, HW], BF16, tag="wvT")
            nc.sync.dma_start(out=wvT_sb, in_=wvT[:, :, :])
            wvb_sb = cp.tile([1, HW], BF16, tag="wvb")
            nc.sync.dma_start(out=wvb_sb, in_=wvb[:, :])
            w_outT_sb = cp.tile([64, H, C], BF16, tag="w_outT")
            nc.sync.dma_start(out=w_outT_sb, in_=w_outT[:, :, :])
            wob_sb = cp.tile([1, C], BF16, tag="wob")
            nc.sync.dma_start(out=wob_sb, in_=wob[:, :])
            zT_sb = cp.tile([128, NJ, TPC], BF16, tag="zT")
            nc.sync.dma_start(out=zT_sb, in_=zT[:, :, :])

            ones128 = cp.tile([1, 128], BF16, tag="ones128")
            nc.vector.memset(ones128, 1.0)
            onest = cp.tile([1, 512], BF16, tag="onest")
            nc.vector.memset(onest, 1.0)

            # ---- persistent activations ----
            xeT_sb = ap.tile([128, KE, TPC], BF16, tag="xeT")
            QT_sb = ap.tile([128, KE, TPC], BF16, tag="QT")
            KT_sb = ap.tile([128, KE, S], BF16, tag="KT")
            V_sb = ap.tile([128, NJ, HW], BF16, tag="V")
            ocat_sb = ap.tile([64, H, TPC], BF16, tag="ocat")
            fin_sb = ap.tile([128, MC, TPC], F32, tag="fin")

            # ---- projection helpers ----
            def emit_xe(m):
                p = ps_mm.tile([128, TPC], F32, tag="mm")
                for t in range(NT):
                    for k in range(KC):
                        nc.tensor.matmul(
                            p[:, t * 512:(t + 1) * 512],
                            w_inT_sb[:, k, m * 128:(m + 1) * 128],
                            xT_sb[:, k, t * 512:(t + 1) * 512],
                            start=(k == 0), stop=(k == KC - 1))
                nc.scalar.activation(
                    out=xeT_sb[:, m, :], in_=p[:, :],
                    func=AF.Identity, bias=b_in_sb[:, m:m + 1], scale=1.0)

            def emit_q(m, dve=False):
                p = ps_mm.tile([128, TPC], F32, tag="mm")
                for t in range(NT):
                    for k in range(KE):
                        nc.tensor.matmul(
                            p[:, t * 512:(t + 1) * 512],
                            wqT_sb[:, k, m * 128:(m + 1) * 128],
                            xeT_sb[:, k, t * 512:(t + 1) * 512],
                            start=(k == 0), stop=(k == KE - 1))
                if dve:
                    nc.vector.tensor_scalar_add(QT_sb[:, m, :], p[:, :],
                                                bq_sb[:, m:m + 1])
                else:
                    nc.scalar.activation(
                        out=QT_sb[:, m, :], in_=p[:, :],
                        func=AF.Identity, bias=bq_sb[:, m:m + 1], scale=1.0)

            def emit_k(m, dve=False):
                for sch in range(S // TPC):
                    p = ps_mm.tile([128, TPC], F32, tag="mm")
                    for t in range(NT):
                        sc = sch * NT + t
                        for k in range(KE):
                            nc.tensor.matmul(
                                p[:, t * 512:(t + 1) * 512],
                                wkT_sb[:, k, m * 128:(m + 1) * 128],
                                ctxT_sb[:, k, sc * 512:(sc + 1) * 512],
                                start=(k == 0), stop=(k == KE - 1))
                    if dve:
                        nc.vector.tensor_scalar_add(
                            KT_sb[:, m, sch * TPC:(sch + 1) * TPC], p[:, :],
                            bk_sb[:, m:m + 1])
                    else:
                        nc.scalar.activation(
                            out=KT_sb[:, m, sch * TPC:(sch + 1) * TPC], in_=p[:, :],
                            func=AF.Identity, bias=bk_sb[:, m:m + 1], scale=1.0)

            def emit_v(st):
                pv = ps_mm.tile([128, HW], F32, tag="mm")
                for k in range(KE):
                    nc.tensor.matmul(
                        pv[:, 0:512],
                        ctxT_sb[:, k, st * 128:(st + 1) * 128],
                        wvT_sb[:, k, 0:512],
                        start=(k == 0), stop=False)
                    nc.tensor.matmul(
                        pv[:, 512:HW],
                        ctxT_sb[:, k, st * 128:(st + 1) * 128],
                        wvT_sb[:, k, 512:HW],
                        start=(k == 0), stop=False)
                nc.tensor.matmul(pv[:, 0:512], ones128[0:1, :], wvb_sb[0:1, 0:512],
                                 start=False, stop=True)
                nc.tensor.matmul(pv[:, 512:HW], ones128[0:1, :], wvb_sb[0:1, 512:HW],
                                 start=False, stop=True)
                nc.scalar.activation(out=V_sb[:, st, :], in_=pv[:, :],
                                     func=AF.Copy, scale=1.0)

            def emit_head(h):
                et, bp = h // 2, 64 * (h % 2)
                oavs = [ps_av.tile([65, 512], F32, tag="av", name=f"oav_{h}_{i}") for i in range(NT)]
                for j in range(NJ):
                    pqk = ps_mm.tile([128, TPC], F32, tag="mm")
                    for t in range(NT):
                        nc.tensor.matmul(
                            pqk[:, t * 512:(t + 1) * 512],
                            KT_sb[bp:bp + 64, et, j * 128:(j + 1) * 128],
                            QT_sb[bp:bp + 64, et, t * 512:(t + 1) * 512],
                            start=True, stop=True)
                    pe_t = pp.tile([128, TPC], BF16, tag="pexp")
                    nc.scalar.activation(out=pe_t[:, :], in_=pqk[:, :],
                                         func=AF.Exp, scale=SCALE)
                    pm_t = pp.tile([128, TPC], BF16, tag="pmask")
                    nc.vector.tensor_mul(pm_t[:, :], pe_t[:, :], zT_sb[:, j, :])
                    for t in range(NT):
                        nc.tensor.matmul(
                            oavs[t][:, :],
                            V_sb[:, j, h * 65:(h + 1) * 65],
                            pm_t[:, t * 512:(t + 1) * 512],
                            start=(j == 0), stop=(j == NJ - 1))
                for t in range(NT):
                    rec = nrm.tile([1, 512], F32, tag="rec")
                    nc.vector.reciprocal(rec[0:1, :], oavs[t][64:65, :])
                    rb = nrm.tile([64, 512], F32, tag="rb")
                    nc.gpsimd.partition_broadcast(rb[:, :], rec[0:1, :])
                    nc.vector.tensor_mul(
                        ocat_sb[0:64, h, t * 512:(t + 1) * 512],
                        oavs[t][0:64, :], rb[:, :])

            # ---- emission schedule: front-load what head 0/1 needs, then
            # interleave the remaining Q/K projections between head pairs so
            # they fill PE slack during the ACT-bound attention phase ----
            for m in range(KE):
                emit_xe(m)
            emit_q(0)
            emit_k(0)
            for st in range(NJ):
                emit_v(st)
            for pair in range(4):
                emit_head(2 * pair)
                emit_head(2 * pair + 1)
                if pair < 3:
                    emit_q(pair + 1)
                    emit_k(pair + 1)

            # ---- proj_out: finT[c, t] ----
            for m in range(MC):
                pf = ps_mm.tile([128, TPC], F32, tag="mm")
                for t in range(NT):
                    for h in range(H):
                        nc.tensor.matmul(
                            pf[:, t * 512:(t + 1) * 512],
                            w_outT_sb[0:64, h, m * 128:(m + 1) * 128],
                            ocat_sb[0:64, h, t * 512:(t + 1) * 512],
                            start=(h == 0), stop=False)
                    nc.tensor.matmul(pf[:, t * 512:(t + 1) * 512],
                                     wob_sb[0:1, m * 128:(m + 1) * 128],
                                     onest[0:1, :], start=False, stop=True)
                nc.vector.tensor_copy(fin_sb[:, m, :], pf[:, :])

            nc.sync.dma_start(out=outT[:, :, :], in_=fin_sb)

    nc.finalize()
    return nc


def get_nc():
    global _NC_CACHE
    if _NC_CACHE is None:
        _NC_CACHE = _build_nc()
    return _NC_CACHE


def _pack(a, p=128):
    """[k*p, f...] -> [p, k, f...] C-contiguous."""
    k = a.shape[0] // p
    return np.ascontiguousarray(
        a.reshape(k, p, *a.shape[1:]).transpose(1, 0, *range(2, a.ndim + 1)))


def build_in_maps(x, context, pad_mask, w_in, b_in, wq, bq, wk, bk, wv, bv,
                  w_out, b_out):
    bf = ml_dtypes.bfloat16
    f32 = np.float32

    w_inT_p = _pack(w_in.T.astype(bf))                    # [128, 2, 512]
    wqT_p = _pack(wq.T.astype(bf))                        # [128, 4, 512]
    wkT_p = _pack(wk.T.astype(bf))
    wvT_aug = np.zeros((E, HW), dtype=bf)
    wvb_row = np.zeros((1, HW), dtype=bf)
    for h in range(H):
        wvT_aug[:, h * 65:h * 65 + 64] = wv.T[:, h * 64:(h + 1) * 64].astype(bf)
        wvb_row[0, h * 65:h * 65 + 64] = bv[h * 64:(h + 1) * 64].astype(bf)
        wvb_row[0, h * 65 + 64] = 1.0
    wvT_p = _pack(wvT_aug)                                # [128, 4, 520]
    w_outT_hm = np.ascontiguousarray(
        w_out.T.reshape(H, 64, C).transpose(1, 0, 2)).astype(bf)  # [64, 8, 256]
    wob_row = b_out[None, :].astype(bf)
    b_in_p = np.ascontiguousarray(b_in.reshape(KE, 128).T).astype(f32)
    bq_p = np.ascontiguousarray(bq.reshape(KE, 128).T).astype(f32)
    bk_p = np.ascontiguousarray(bk.reshape(KE, 128).T).astype(f32)

    z = (~pad_mask).astype(bf)                            # [B, T, S]

    in_maps = []
    for c in range(NCORES):
        b, th = c // 2, c % 2
        t0 = th * TPC
        xT_p = _pack(np.ascontiguousarray(x[b, t0:t0 + TPC, :].T).astype(bf))
        ctxT_p = _pack(np.ascontiguousarray(context[b].T).astype(bf))
        zT_p = _pack(np.ascontiguousarray(z[b, t0:t0 + TPC, :].T))
        in_maps.append({
            "xT": xT_p, "ctxT": ctxT_p, "zT": zT_p,
            "w_inT": w_inT_p, "wqT": wqT_p, "wkT": wkT_p, "wvT": wvT_p,
            "wvb": wvb_row, "w_outT": w_outT_hm, "wob": wob_row,
            "b_in": b_in_p, "bq": bq_p, "bk": bk_p,
        })
    return in_maps


def assemble_output(results):
    out = np.empty((B, T, C), dtype=np.float32)
    for c in range(NCORES):
        b, th = c // 2, c % 2
        t0 = th * TPC
        arr = results[c]["outT"]                          # [128, 2, 1024]
        ct = arr.transpose(1, 0, 2).reshape(C, TPC)       # [256, 1024]
        out[b, t0:t0 + TPC, :] = ct.T
    return out


def run(in_maps, **kw):
    return run_bass_kernel_spmd(get_nc(), in_maps, core_ids=list(range(NCORES)),
                                **kw)


def kernel(**inputs):
    in_maps = build_in_maps(
        np.asarray(inputs["x"]), np.asarray(inputs["context"]),
        np.asarray(inputs["pad_mask"]), np.asarray(inputs["w_in"]),
        np.asarray(inputs["b_in"]), np.asarray(inputs["wq"]),
        np.asarray(inputs["bq"]), np.asarray(inputs["wk"]),
        np.asarray(inputs["bk"]), np.asarray(inputs["wv"]),
        np.asarray(inputs["bv"]), np.asarray(inputs["w_out"]),
        np.asarray(inputs["b_out"]))
    res = run(in_maps)
    return assemble_output(res.results)
